# revision 10
# baseline (speedup 1.0000x reference)
"""Dual scaled-dot-product attention — TRN2 Bass kernel (v5).

Problem (per full input):
  B=64, L1=L2=1024, F1=F2=A=128
  q = f1 @ W1^T + b1; k = f2 @ W2^T + b2
  S = q @ k^T / sqrt(A); masked = where(mask==0, -1e9, S)
  out1 = softmax(masked, axis=1)^T-weighted sum of f1   [B, L2, F1]
  out2 = softmax(masked, axis=2)-weighted sum of f2     [B, L1, F2]

Sharding: data-parallel over batch across 8 cores (8 batches/core).

Design:
  - All 128x128 transposes are regular matmuls against a bf16 identity
    (~81ns/block, keeps the PE HAM-warm at 2.4GHz) instead of
    transpose-mode (~275ns/block, HAM-cold).
  - exp(S) multiplies the raw int32 mask directly on DVE/GpSimd
    (engines convert to fp32 internally) — no separate cast pass.
  - Softmax denominators ride free: a ones column appended to the bf16
    feature copies accumulates colsum/rowsum in PSUM column F of the
    U/V2 matmuls.
  - DMA rings split: SP carries only the mask stream (4MB/batch,
    3 tiles ahead, 6 buffers); feature loads + output stores on the
    ACT ring.
  - The U phase (out1) of batch b is interleaved into batch b+1's tile
    loop (one m-tile pair per odd iteration), removing the end-of-batch
    PE convoy that idled ACT/DVE.
  - PSUM: S/qk double-buffered (2x2 banks), E^T staging single
    (2 banks), U/V2 pair accumulators (2x1 bank).
"""

from contextlib import ExitStack

import numpy as np

import concourse.bass as bass
import concourse.tile as tile
from concourse import bacc
from concourse import mybir
from concourse.bass_utils import run_bass_kernel_spmd
from concourse.masks import make_identity

B, L, F, A = 64, 1024, 128, 128
NCORES = 8
BPC = B // NCORES          # batches per core
P = 128                    # SBUF partitions
NT = L // P                # 8 row-tiles per batch
SCALE = float(1.0 / np.sqrt(np.float32(A)))

F32 = mybir.dt.float32
BF16 = mybir.dt.bfloat16
I32 = mybir.dt.int32
EXP = mybir.ActivationFunctionType.Exp
IDENT = mybir.ActivationFunctionType.Identity


def _body(ctx, tc, f1h, f2h, mh, w1h, b1h, w2h, b2h, o1h, o2h, bpc,
          parts=("compute",)):
    nc = tc.nc
    dma_only = "dma_only" in parts

    consts = ctx.enter_context(tc.tile_pool(name="consts", bufs=1))
    fpool = ctx.enter_context(tc.tile_pool(name="fpool", bufs=2))
    ftpool = ctx.enter_context(tc.tile_pool(name="ftpool", bufs=3))
    mpool = ctx.enter_context(tc.tile_pool(name="mpool", bufs=6))
    epool = ctx.enter_context(tc.tile_pool(name="epool", bufs=4))
    e0pool = ctx.enter_context(tc.tile_pool(name="e0pool", bufs=2))
    opool = ctx.enter_context(tc.tile_pool(name="opool", bufs=3))
    rpool = ctx.enter_context(tc.tile_pool(name="rpool", bufs=4))

    ppbig = ctx.enter_context(tc.tile_pool(name="ppbig", bufs=2, space="PSUM"))
    ppstg = ctx.enter_context(tc.tile_pool(name="ppstg", bufs=2, space="PSUM"))
    ppuv = ctx.enter_context(tc.tile_pool(name="ppuv", bufs=2, space="PSUM"))

    # ---- one-time constants ----
    id32 = consts.tile([P, P], F32)
    make_identity(nc, id32)
    id16 = consts.tile([P, P], BF16)
    make_identity(nc, id16)

    w1n = consts.tile([P, P], F32)
    w2n = consts.tile([P, P], F32)
    nc.sync.dma_start(out=w1n, in_=w1h[:, :])
    nc.sync.dma_start(out=w2n, in_=w2h[:, :])
    b1s = consts.tile([P, 1], F32)
    b2s = consts.tile([P, 1], F32)
    nc.sync.dma_start(out=b1s, in_=b1h.ap().rearrange("(a o) -> a o", o=1))
    nc.sync.dma_start(out=b2s, in_=b2h.ap().rearrange("(a o) -> a o", o=1))

    w1T = consts.tile([P, P], BF16)
    w2T = consts.tile([P, P], BF16)
    wstg = ppstg.tile([P, NT // 2, P], F32, tag="stg")
    nc.tensor.matmul(out=wstg[:, 0, :], lhsT=w1n, rhs=id32,
                     start=True, stop=True)
    nc.tensor.matmul(out=wstg[:, 1, :], lhsT=w2n, rhs=id32,
                     start=True, stop=True)
    nc.vector.tensor_copy(out=w1T, in_=wstg[:, 0, :])
    nc.vector.tensor_copy(out=w2T, in_=wstg[:, 1, :])

    def prologue(b):
        """Loads (ACT ring), feature transposes, bf16 copies, q/k."""
        st = {"mts": {}}
        f1n = fpool.tile([P, NT, F], F32, tag="f1n", name="f1n")
        f2n = fpool.tile([P, NT, F], F32, tag="f2n", name="f2n")
        f1r = f1h[b].rearrange("(i p) d -> p i d", p=P)
        f2r = f2h[b].rearrange("(i p) d -> p i d", p=P)
        nc.sync.dma_start(out=f1n, in_=f1r)
        nc.sync.dma_start(out=f2n, in_=f2r)

        if dma_only:
            st["f1n"], st["f2n"] = f1n, f2n
            st["o1t"] = opool.tile([P, NT, F], F32, tag="o1", name="o1t")
            st["o2t"] = opool.tile([P, NT, F], F32, tag="o2", name="o2t")
            nc.vector.memset(st["o1t"][:, 0, 0:1], 0.0)
            nc.vector.memset(st["o2t"][:, 0, 0:1], 0.0)
            return st

        f1p = ftpool.tile([P, NT, F + 1], BF16, tag="f1p", name="f1p")
        f2p = ftpool.tile([P, NT, F + 1], BF16, tag="f2p", name="f2p")
        nc.vector.memset(f1p[:, :, F:F + 1], 1.0)
        nc.vector.memset(f2p[:, :, F:F + 1], 1.0)
        nc.scalar.copy(out=f1p[:, :, 0:F], in_=f1n)
        nc.gpsimd.tensor_copy(out=f2p[:, :, 0:F], in_=f2n)

        f1T = ftpool.tile([P, L], BF16, tag="f1T", name="f1T")
        f2T = ftpool.tile([P, L], BF16, tag="f2T", name="f2T")
        for src_, dst, eng in ((f1p, f1T, "act"), (f2p, f2T, "dve")):
            for g in range(2):
                stg = ppstg.tile([P, NT // 2, P], F32, tag="stg",
                                 name="fstg")
                for q in range(NT // 2):
                    nc.tensor.matmul(out=stg[:, q, :],
                                     lhsT=src_[:, g * 4 + q, 0:F],
                                     rhs=id16, start=True, stop=True)
                flat = stg.rearrange("p a c -> p (a c)")
                half = dst[:, g * 512:(g + 1) * 512]
                if eng == "act":
                    nc.scalar.copy(out=half, in_=flat)
                else:
                    nc.vector.tensor_copy(out=half, in_=flat)

        qT = ftpool.tile([P, L], BF16, tag="qT", name="qT")
        kT = ftpool.tile([P, L], BF16, tag="kT", name="kT")
        for wT, fT, bs, dst, eng in ((w1T, f1T, b1s, qT, "act"),
                                     (w2T, f2T, b2s, kT, "dve")):
            pp = ppbig.tile([P, L], F32, tag="qk", name="qkp")
            for g in range(2):
                nc.tensor.matmul(out=pp[:, g * 512:(g + 1) * 512],
                                 lhsT=wT,
                                 rhs=fT[:, g * 512:(g + 1) * 512],
                                 start=True, stop=True)
            if eng == "act":
                nc.scalar.activation(out=dst, in_=pp, func=IDENT,
                                     bias=bs, scale=1.0)
            else:
                nc.vector.tensor_scalar_add(out=dst, in0=pp, scalar1=bs)

        st["f1p"], st["f2p"], st["qT"], st["kT"] = f1p, f2p, qT, kT
        st["e0s"] = [e0pool.tile([P, L], BF16, tag=f"E0_{i}", name=f"E0_{i}")
                     for i in range(NT)]
        st["e0t"] = e0pool.tile([P, NT, L], BF16, tag="E0T", name="E0T")
        st["o1t"] = opool.tile([P, NT, F], F32, tag="o1", name="o1t")
        st["o2t"] = opool.tile([P, NT, F], F32, tag="o2", name="o2t")
        return st

    def mask_dma(b, st, i):
        """Mask tile load on the SP ring (its own stream)."""
        mt = mpool.tile([P, L], I32, tag="mask", name="mt")
        nc.sync.dma_start(out=mt, in_=mh[b, i * P:(i + 1) * P, :])
        st["mts"][i] = mt

    def score_tile(b, st, i):
        """S matmuls (PE), exp (ACT), mask mul (DVE/Pool)."""
        mt = st["mts"].pop(i)
        sp = ppbig.tile([P, L], F32, tag="qk", name="sp")
        for g in range(2):
            nc.tensor.matmul(out=sp[:, g * 512:(g + 1) * 512],
                             lhsT=st["qT"][:, i * P:(i + 1) * P],
                             rhs=st["kT"][:, g * 512:(g + 1) * 512],
                             start=True, stop=True)
        et = epool.tile([P, L], BF16, tag="et", name="et")
        nc.scalar.activation(out=et, in_=sp, func=EXP, scale=SCALE)
        if i % 3 == 2:
            nc.gpsimd.tensor_mul(out=st["e0s"][i], in0=et, in1=mt)
        else:
            nc.vector.tensor_mul(out=st["e0s"][i], in0=et, in1=mt)

    def transpose_tile(b, st, i):
        """E^T blocks via identity matmul (f32 PSUM) + one wide
        PSUM->SBUF bf16 copy (alternating ACT/DVE)."""
        for g in range(2):
            stg = ppstg.tile([P, NT // 2, P], F32, tag="stg", name="estg")
            for q in range(NT // 2):
                j = g * 4 + q
                nc.tensor.matmul(out=stg[:, q, :],
                                 lhsT=st["e0s"][i][:, j * P:(j + 1) * P],
                                 rhs=id16, start=True, stop=True)
            flat = stg.rearrange("p a c -> p (a c)")
            dst = st["e0t"][:, g * 4:(g + 1) * 4, i * P:(i + 1) * P]
            if (i + g) % 2 == 0:
                nc.scalar.copy(out=dst, in_=flat)
            else:
                nc.vector.tensor_copy(out=dst, in_=flat)

    def o_store(b, st, which, half):
        oh = o1h if which == "o1" else o2h
        ot = st["o1t"] if which == "o1" else st["o2t"]
        hr = oh[b].rearrange("(j p) f -> p j f", p=P)
        if half == 0:
            nc.gpsimd.dma_start(out=hr[:, 0:4, :], in_=ot[:, 0:4, :])
        else:
            nc.gpsimd.dma_start(out=hr[:, 4:NT, :], in_=ot[:, 4:NT, :])

    def v2_pair(b, st, i0):
        """out2 rows for l_tiles i0, i0+1 sharing one PSUM bank; one
        reciprocal per pair; scale/move split ACT/DVE."""
        vp = ppuv.tile([P, 2, F + 1], F32, tag="uv", name="vp")
        for s in range(2):
            i = i0 + s
            for j in range(NT):
                nc.tensor.matmul(out=vp[:, s, :],
                                 lhsT=st["e0t"][:, j, i * P:(i + 1) * P],
                                 rhs=st["f2p"][:, j, :],
                                 start=(j == 0), stop=(j == NT - 1))
        rv = rpool.tile([P, 2], F32, tag="r", name="rv")
        nc.vector.reciprocal(
            out=rv, in_=vp[:, :, F:F + 1].rearrange("p a o -> p (a o)"))
        nc.scalar.mul(out=st["o2t"][:, i0, :], in_=vp[:, 0, 0:F],
                      mul=rv[:, 0:1])
        nc.vector.tensor_scalar_mul(out=st["o2t"][:, i0 + 1, :],
                                    in0=vp[:, 1, 0:F], scalar1=rv[:, 1:2])

    def u_pair(b, st, j0):
        """out1 columns for m_tiles j0, j0+1, same pairing."""
        up = ppuv.tile([P, 2, F + 1], F32, tag="uv", name="up")
        for s in range(2):
            j = j0 + s
            for i in range(NT):
                nc.tensor.matmul(out=up[:, s, :],
                                 lhsT=st["e0s"][i][:, j * P:(j + 1) * P],
                                 rhs=st["f1p"][:, i, :],
                                 start=(i == 0), stop=(i == NT - 1))
        ru = rpool.tile([P, 2], F32, tag="r", name="ru")
        nc.vector.reciprocal(
            out=ru, in_=up[:, :, F:F + 1].rearrange("p a o -> p (a o)"))
        nc.vector.tensor_scalar_mul(out=st["o1t"][:, j0, :],
                                    in0=up[:, 0, 0:F], scalar1=ru[:, 0:1])
        nc.scalar.mul(out=st["o1t"][:, j0 + 1, :], in_=up[:, 1, 0:F],
                      mul=ru[:, 1:2])

    if dma_only:
        for b in range(bpc):
            st = prologue(b)
            for i in range(NT):
                mask_dma(b, st, i)
            nc.gpsimd.dma_start(
                out=o1h[b].rearrange("(j p) f -> p j f", p=P),
                in_=st["o1t"])
            nc.gpsimd.dma_start(
                out=o2h[b].rearrange("(i p) f -> p i f", p=P),
                in_=st["o2t"])
        return

    # Pipeline: mask DMAs 3 tiles ahead on their own ring; v2 one pair
    # behind its transposes; U phase of batch b-1 interleaved into odd
    # iterations of batch b; next batch's prologue at i==5.
    states = {0: prologue(0)}
    for i in range(3):
        mask_dma(0, states[0], i)
    score_tile(0, states[0], 0)
    prev = None
    for b in range(bpc):
        st = states.pop(b)
        for i in range(NT):
            if i + 3 < NT:
                mask_dma(b, st, i + 3)
            elif b + 1 < bpc:
                if i + 3 == NT:
                    states[b + 1] = prologue(b + 1)
                mask_dma(b + 1, states[b + 1], i + 3 - NT)
            if i + 1 < NT:
                score_tile(b, st, i + 1)
            if i > 0 and i % 2 == 0:
                v2_pair(b, st, i - 2)       # tiles i-2, i-1 complete
            if i == 6:
                o_store(b, st, "o2", 0)     # rows 0..511 final
            transpose_tile(b, st, i)
            if prev is not None and i % 2 == 1:
                pb, pst = prev
                j0 = i - 1                  # 0, 2, 4, 6
                u_pair(pb, pst, j0)
                if j0 == 2:
                    o_store(pb, pst, "o1", 0)
                elif j0 == 6:
                    o_store(pb, pst, "o1", 1)
        if b + 1 < bpc:
            score_tile(b + 1, states[b + 1], 0)
        v2_pair(b, st, NT - 2)
        o_store(b, st, "o2", 1)
        prev = (b, st)
    # drain last batch's U phase
    pb, pst = prev
    for j0 in range(0, NT, 2):
        u_pair(pb, pst, j0)
        if j0 == 2:
            o_store(pb, pst, "o1", 0)
    o_store(pb, pst, "o1", 1)


def build_nc(bpc: int = BPC, repeat: int = 1,
             parts=("compute",)) -> bass.Bass:
    nc = bacc.Bacc()
    f1h = nc.dram_tensor("feature1", [bpc, L, F], F32, kind="ExternalInput")
    f2h = nc.dram_tensor("feature2", [bpc, L, F], F32, kind="ExternalInput")
    mh = nc.dram_tensor("mask", [bpc, L, L], I32, kind="ExternalInput")
    w1h = nc.dram_tensor("W1", [A, F], F32, kind="ExternalInput")
    b1h = nc.dram_tensor("b1", [A], F32, kind="ExternalInput")
    w2h = nc.dram_tensor("W2", [A, F], F32, kind="ExternalInput")
    b2h = nc.dram_tensor("b2", [A], F32, kind="ExternalInput")
    o1h = nc.dram_tensor("out1", [bpc, L, F], F32, kind="ExternalOutput")
    o2h = nc.dram_tensor("out2", [bpc, L, F], F32, kind="ExternalOutput")

    with tile.TileContext(nc) as tc:
        with ExitStack() as ctx:
            if repeat == 1:
                _body(ctx, tc, f1h, f2h, mh, w1h, b1h, w2h, b2h, o1h, o2h,
                      bpc, parts=parts)
            else:
                with tc.For_i(0, repeat, 1):
                    _body(ctx, tc, f1h, f2h, mh, w1h, b1h, w2h, b2h, o1h,
                          o2h, bpc, parts=parts)
    nc.compile()
    return nc


_NC_CACHE: dict = {}


def _get_nc() -> bass.Bass:
    if "nc" not in _NC_CACHE:
        _NC_CACHE["nc"] = build_nc(BPC)
    return _NC_CACHE["nc"]


def _in_maps(feature1, feature2, mask, W1, b1, W2, b2):
    f1 = np.ascontiguousarray(np.asarray(feature1, dtype=np.float32))
    f2 = np.ascontiguousarray(np.asarray(feature2, dtype=np.float32))
    mk = np.ascontiguousarray(np.asarray(mask, dtype=np.int32))
    w1 = np.ascontiguousarray(np.asarray(W1, dtype=np.float32))
    w2 = np.ascontiguousarray(np.asarray(W2, dtype=np.float32))
    bb1 = np.ascontiguousarray(np.asarray(b1, dtype=np.float32))
    bb2 = np.ascontiguousarray(np.asarray(b2, dtype=np.float32))
    maps = []
    for c in range(NCORES):
        sl = slice(c * BPC, (c + 1) * BPC)
        maps.append({
            "feature1": np.ascontiguousarray(f1[sl]),
            "feature2": np.ascontiguousarray(f2[sl]),
            "mask": np.ascontiguousarray(mk[sl]),
            "W1": w1, "b1": bb1, "W2": w2, "b2": bb2,
        })
    return maps


def run(feature1, feature2, mask, W1, b1, W2, b2, **spmd_kwargs):
    """Run on all 8 cores; returns (out1, out2, BassKernelResults)."""
    nc = _get_nc()
    maps = _in_maps(feature1, feature2, mask, W1, b1, W2, b2)
    res = run_bass_kernel_spmd(nc, maps, core_ids=list(range(NCORES)),
                               **spmd_kwargs)
    out1 = np.concatenate([res.results[c]["out1"] for c in range(NCORES)],
                          axis=0)
    out2 = np.concatenate([res.results[c]["out2"] for c in range(NCORES)],
                          axis=0)
    return out1, out2, res


def kernel(feature1, feature2, mask, W1, b1, W2, b2):
    out1, out2, _ = run(feature1, feature2, mask, W1, b1, W2, b2)
    return out1, out2
